# revision 15
# baseline (speedup 1.0000x reference)
# Self-contained Trainium2 Bass kernel for 16-head MultiHeadAttention
# (B=4, L=2048, HIDDEN=1024, 16 heads x d_k=64), sharded 2 heads per core
# across 8 NeuronCores (tensor-parallel on heads; every core sees all tokens).
#
# Per-core plan (all matmuls bf16 with fp32 PSUM accumulation):
#   x^T and W^T are pre-transposed on the host (plain strided DMA loads,
#   no on-device XBAR transposes for them).
#   Q^T,K^T = W^T-stationary matmuls -> [128 (2 heads x 64), 8192] bf16 (+bias)
#   V^T -> XBAR-transpose -> V natural [token-part, 2x(64+ones)] (ones col
#          folds the softmax denominator into the AV matmul)
#   S^T tile = K_tile @ Q^T  (row-tiled pairs: head0 on PE rows 0-63, head1 on
#          rows 64-127 run concurrently via tile_position)
#   P^T = exp(S^T/8) on ScalarE straight from PSUM (no max subtraction)
#   att^T[65, Lq] += V_aug^T @ P^T  (V stationary; row 64 = denominator)
#   out = att^T[0:64] * broadcast(1/denominator)  [reciprocal_approx_fast]
#
# Scheduling: ScalarE exp is the bottleneck (~1us per [128,1024] tile, 256
# tiles). All projection work for batch b+1 is chopped into ~1.7us filler
# pieces (half proj groups, xt loads, V transposes) and emitted between the
# lk iterations of batch b's attention so both PE and ScalarE stay busy;
# a warmup matmul burst un-throttles the PE HAM clock gate at start.

import numpy as np

NUM_HEADS = 16
HIDDEN = 1024
D_K = 64
B = 4
L = 2048
N_CORES = 8
HPC = NUM_HEADS // N_CORES      # heads per core = 2
OPC = HPC * D_K                 # output dims per core = 128

P = 128
T = B * L                       # 8192 tokens
KT = HIDDEN // P                # 8 contraction tiles
TCH = 1024                      # token chunk for x projection
NCH = T // TCH                  # 8 chunks
LKT = L // P                    # 16 key tiles per batch
QC = 512                        # query chunk (one PSUM bank)
LQC = L // QC                   # 4 query chunks per batch

_CACHE = {}


def _build_nc(reps=1):
    import contextlib

    import concourse.bacc as bacc
    import concourse.mybir as mybir
    import concourse.tile as tile

    dt = mybir.dt
    AF = mybir.ActivationFunctionType
    ALU = mybir.AluOpType

    nc = bacc.Bacc(None, target_bir_lowering=False, debug=False)

    # x^T, bf16-cast + transposed on the host: [HIDDEN, T]
    xT = nc.declare_dram_parameter("xT", [HIDDEN, T], dt.bfloat16, isOutput=False)
    # weights pre-transposed on host to [p, kt, c] = W[c, kt*128+p]
    wq = nc.declare_dram_parameter("wq", [P, KT, P], dt.bfloat16, isOutput=False)
    wk = nc.declare_dram_parameter("wk", [P, KT, P], dt.bfloat16, isOutput=False)
    wv = nc.declare_dram_parameter("wv", [P, KT, P], dt.bfloat16, isOutput=False)
    bq = nc.declare_dram_parameter("bq", [P, 1], dt.float32, isOutput=False)
    bk = nc.declare_dram_parameter("bk", [P, 1], dt.float32, isOutput=False)
    bv = nc.declare_dram_parameter("bv", [P, 1], dt.float32, isOutput=False)
    out = nc.declare_dram_parameter("out", [HPC, D_K, T], dt.float32, isOutput=True)

    with tile.TileContext(nc) as tc:
        with (
            tc.tile_pool(name="const", bufs=1) as const,
            tc.tile_pool(name="persist", bufs=1) as persist,
            tc.tile_pool(name="xtp", bufs=2) as xtp,
            tc.tile_pool(name="vtp", bufs=2) as vtp,
            tc.tile_pool(name="ptp", bufs=8) as ptp,
            tc.tile_pool(name="fin", bufs=2) as fin,
            # PSUM budget (8 banks): proj halves + scores share "mm" 3x2,
            # attended accumulators 2x1.
            tc.tile_pool(name="mm", bufs=3, space="PSUM") as mmp,
            tc.tile_pool(name="avp", bufs=2, space="PSUM") as avp,
        ):
            # (No HAM warmup burst: it runs serially at cold rate before the
            # first projection and costs ~10us while saving ~1.5us — the
            # startup projections warm the clock gate themselves.)

            # --- weights (host-transposed): straight DMA loads ---
            wts = []
            bts = []
            for nm, wparam, bparam in (("q", wq, bq), ("k", wk, bk), ("v", wv, bv)):
                wt = const.tile([P, KT, P], dt.bfloat16, tag=f"wt{nm}")
                nc.sync.dma_start(out=wt[:], in_=wparam[:])
                bt = const.tile([P, 1], dt.float32, tag=f"b{nm}")
                nc.sync.dma_start(out=bt[:], in_=bparam[:])
                wts.append(wt)
                bts.append(bt)

            # --- persistent activations ---
            qT = persist.tile([P, T], dt.bfloat16, tag="qT")
            kT = persist.tile([P, T], dt.bfloat16, tag="kT")
            # V natural layout + ones columns: [tok-part, tok-tile, 2*(64+1)]
            vaug = persist.tile([P, T // P, 2 * (D_K + 1)], dt.bfloat16, tag="vaug")
            nc.vector.memset(vaug[:, :, D_K:D_K + 1], 1.0)
            nc.vector.memset(vaug[:, :, 2 * D_K + 1:2 * D_K + 2], 1.0)

            ctx = dict(nc=nc, dt=dt, AF=AF, ALU=ALU, qT=qT, kT=kT, vaug=vaug,
                       wts=wts, bts=bts, xT=xT, out=out,
                       mmp=mmp, avp=avp, xtp=xtp, vtp=vtp, ptp=ptp, fin=fin)

            # For timing runs (reps>1) the whole per-call body loops on-device.
            rep_ctx = tc.For_i(0, reps, 1) if reps > 1 else contextlib.nullcontext()
            with rep_ctx:
                # Startup: K of chunks 0-1, first Q half, and all of V (the
                # AV accumulation consumes vaug from lk=0, so V can't lag).
                xt0 = _emit_xt_load(ctx, 0, split=True)
                xt1 = _emit_xt_load(ctx, 1, split=True)
                for h2 in range(2):
                    _emit_proj_half(ctx, 0, xt0, 1, h2)   # K chunk0
                for h2 in range(2):
                    _emit_proj_half(ctx, 1, xt1, 1, h2)   # K chunk1
                _emit_proj_half(ctx, 0, xt0, 0, 0)        # Q chunk0 first half
                for h2 in range(2):
                    _emit_proj_half(ctx, 0, xt0, 2, h2)   # V chunk0
                _emit_vtrans(ctx, 0, 0)
                _emit_vtrans(ctx, 0, 1)
                for h2 in range(2):
                    _emit_proj_half(ctx, 1, xt1, 2, h2)   # V chunk1
                _emit_vtrans(ctx, 1, 0)
                _emit_vtrans(ctx, 1, 1)

                # Remaining batch-0 Q halves become early fillers (their
                # deadlines are qc1/qc2/qc3 starts).
                fillers = []
                fillers.append(lambda: _emit_proj_half(ctx, 0, xt0, 0, 1))
                fillers.append(lambda: _emit_proj_half(ctx, 1, xt1, 0, 0))
                fillers.append(lambda: _emit_proj_half(ctx, 1, xt1, 0, 1))

                for b in range(B):
                    if b < B - 1:
                        # projections for batch b+1 as fillers in batch b;
                        # V first: its multi-engine tail (bias -> transpose ->
                        # vaug copies) must be done well before batch b+1's
                        # AV accumulation starts consuming vaug.
                        c0, c1 = 2 * b + 2, 2 * b + 3
                        xta = [None, None]

                        def mk(fn, *a):
                            return lambda: fn(ctx, *a)

                        def mk_xt(i, ch):
                            def f():
                                xta[i] = _emit_xt_load(ctx, ch)
                            return f

                        def mk_proj(i, ch, idx, h2):
                            return lambda: _emit_proj_half(ctx, ch, xta[i], idx, h2)

                        for i, ch in ((0, c0), (1, c1)):
                            fillers.append(mk_xt(i, ch))
                            fillers.append(mk_proj(i, ch, 2, 0))
                            fillers.append(mk_proj(i, ch, 2, 1))
                            fillers.append(mk(_emit_vtrans, ch, 0))
                            fillers.append(mk(_emit_vtrans, ch, 1))
                        for i, ch in ((0, c0), (1, c1)):
                            fillers.append(mk_proj(i, ch, 1, 0))
                            fillers.append(mk_proj(i, ch, 1, 1))
                            fillers.append(mk_proj(i, ch, 0, 0))
                            fillers.append(mk_proj(i, ch, 0, 1))

                    # distribute pending fillers over this batch's 64 units
                    nf = len(fillers)
                    sched = {}
                    for i, f in enumerate(fillers):
                        sched.setdefault(min(63, i * 64 // max(nf, 1)), []).append(f)
                    fillers = []
                    _emit_attention_batch(ctx, b, sched)

    nc.compile()
    return nc


def _emit_xt_load(ctx, ch, split=False):
    """Load x^T for a token chunk. split=True (startup only) issues
    per-ktile DMAs so the k=0 projection matmul starts after 1/8 of the
    chunk arrives; steady-state chunks use one DMA to keep the Sync
    queue short."""
    nc, dt, xT, xtp = ctx["nc"], ctx["dt"], ctx["xT"], ctx["xtp"]
    t0 = ch * TCH
    xt = xtp.tile([P, KT, TCH], dt.bfloat16, tag="xt")
    if split:
        for k in range(KT):
            nc.sync.dma_start(
                out=xt[:, k, :],
                in_=xT[k * P:(k + 1) * P, t0:t0 + TCH],
            )
    else:
        src = xT[:, t0:t0 + TCH].rearrange("(k p) t -> p k t", p=P)
        nc.sync.dma_start(out=xt[:], in_=src)
    return xt


def _emit_proj_half(ctx, ch, xt, idx, h2):
    """Half (512 tokens) of one chunk's projection through Wq/Wk/Wv."""
    nc, dt = ctx["nc"], ctx["dt"]
    mmp = ctx["mmp"]
    wts, bts = ctx["wts"], ctx["bts"]
    t0 = ch * TCH + h2 * QC
    ps = mmp.tile([P, QC], dt.float32, tag="mm")
    for k in range(KT):
        nc.tensor.matmul(
            ps[:],
            lhsT=wts[idx][:, k, :],
            rhs=xt[:, k, h2 * QC:(h2 + 1) * QC],
            start=(k == 0),
            stop=(k == KT - 1),
        )
    if idx < 2:
        dest = ctx["qT"] if idx == 0 else ctx["kT"]
        nc.vector.tensor_scalar_add(
            out=dest[:, t0:t0 + QC], in0=ps[:], scalar1=bts[idx][:]
        )
    else:
        vtp = ctx["vtp"]
        vt = ctx.setdefault("vt_tiles", {})
        if ch not in vt:
            vt[ch] = vtp.tile([P, TCH], dt.bfloat16, tag="vt", name=f"vt{ch}")
        nc.vector.tensor_scalar_add(
            out=vt[ch][:, h2 * QC:(h2 + 1) * QC], in0=ps[:], scalar1=bts[idx][:]
        )


def _emit_vtrans(ctx, ch, half):
    """Transpose half a chunk of V^T into the natural-layout vaug tile."""
    nc, dt = ctx["nc"], ctx["dt"]
    vtp, vaug = ctx["vtp"], ctx["vaug"]
    vt = ctx["vt_tiles"][ch]
    for j in range(half * (TCH // P // 2), (half + 1) * (TCH // P // 2)):
        vnt = vtp.tile([P, P], dt.bfloat16, tag="vnt")
        nc.sync.dma_start_transpose(vnt[:], vt[:, j * P:(j + 1) * P])
        tt = ch * (TCH // P) + j
        nc.vector.tensor_copy(out=vaug[:, tt, 0:D_K], in_=vnt[:, 0:D_K])
        nc.vector.tensor_copy(
            out=vaug[:, tt, D_K + 1:2 * D_K + 1], in_=vnt[:, D_K:2 * D_K]
        )
    if half == 1:
        del ctx["vt_tiles"][ch]


def _emit_attention_batch(ctx, b, fill_at=None):
    """Scores -> exp -> AV -> normalize for one batch (64 (qc,lk) units).

    The scores matmuls run two units ahead of the exp/AV chain so the exp
    stream on ScalarE has a buffered tile while PE chews through a filler
    (projection work for the next batch, ~1.7us pieces via fill_at:
    {unit_index: [callables]}).
    """
    import numpy as np
    nc, dt, AF = ctx["nc"], ctx["dt"], ctx["AF"]
    qT, kT, vaug = ctx["qT"], ctx["kT"], ctx["vaug"]
    mmp, avp, ptp = ctx["mmp"], ctx["avp"], ctx["ptp"]
    fill_at = fill_at or {}

    def emit_scores(i):
        cq, lk = divmod(i, LKT)
        qs = b * L + cq * QC
        ks = b * L + lk * P
        st = mmp.tile([P, 2, QC], dt.float32, tag="mm", name=f"st{b}_{i}")
        nc.tensor.matmul(
            st[:, 0, :], lhsT=kT[0:D_K, ks:ks + P],
            rhs=qT[0:D_K, qs:qs + QC],
            start=True, stop=True, tile_position=(0, 0),
        )
        nc.tensor.matmul(
            st[:, 1, :], lhsT=kT[D_K:P, ks:ks + P],
            rhs=qT[D_K:P, qs:qs + QC],
            start=True, stop=True, tile_position=(64, 0),
        )
        return st

    sts = {0: emit_scores(0), 1: emit_scores(1)}
    avs01 = None
    for i in range(LQC * LKT):
        cq, lk = divmod(i, LKT)
        if lk == 0:
            avs01 = (avp.tile([P, QC], dt.float32, tag="av", name=f"av0_{b}_{cq}"),
                     avp.tile([P, QC], dt.float32, tag="av", name=f"av1_{b}_{cq}"))
        av0, av1 = avs01
        st = sts.pop(i)
        pt = ptp.tile([P, 2, QC], dt.bfloat16, tag="pt", name=f"pt{b}_{i}")
        nc.scalar.activation(
            out=pt[:], in_=st[:], func=AF.Exp, scale=1.0 / np.sqrt(D_K),
        )
        ltile = b * LKT + lk
        nc.tensor.matmul(
            av0[:D_K + 1, :], lhsT=vaug[:, ltile, 0:D_K + 1],
            rhs=pt[:, 0, :],
            start=(lk == 0), stop=(lk == LKT - 1),
        )
        nc.tensor.matmul(
            av1[:D_K + 1, :],
            lhsT=vaug[:, ltile, D_K + 1:2 * (D_K + 1)],
            rhs=pt[:, 1, :],
            start=(lk == 0), stop=(lk == LKT - 1),
        )
        if i + 2 < LQC * LKT:
            sts[i + 2] = emit_scores(i + 2)
        for f in fill_at.get(i, ()):
            f()
        if lk == LKT - 1:
            _emit_finalize(ctx, b, cq, av0, av1)


def _emit_finalize(ctx, b, cq, av0, av1):
    nc, dt, ALU = ctx["nc"], ctx["dt"], ctx["ALU"]
    fin, out = ctx["fin"], ctx["out"]
    qs = b * L + cq * QC
    for h, av in ((0, av0), (1, av1)):
        # evict PSUM->SBUF first so the accumulator bank frees immediately
        avs = fin.tile([D_K, QC], dt.float32, tag="avs")
        nc.vector.tensor_copy(out=avs[:], in_=av[0:D_K, :])
        # move the denominator row to a partition-0 tile:
        # reciprocal_approx_fast (custom DVE op) only works at base partition 0
        d0 = fin.tile([1, QC], dt.float32, tag="d0")
        nc.vector.tensor_copy(out=d0[:], in_=av[D_K:D_K + 1, :])
        rc = fin.tile([1, QC], dt.float32, tag="rc")
        nc.vector.reciprocal_approx_fast(out=rc[:], in_=d0[:])
        bc = fin.tile([D_K, QC], dt.float32, tag="bc")
        nc.gpsimd.partition_broadcast(bc[:], rc[:])
        osb = fin.tile([D_K, QC], dt.float32, tag="osb")
        nc.vector.tensor_tensor(osb[:], avs[:], bc[:], ALU.mult)
        nc.gpsimd.dma_start(out=out[h, :, qs:qs + QC], in_=osb[:])


def get_nc(reps=1, **kw):
    key = f"nc{reps}-{sorted(kw.items())}"
    if key not in _CACHE:
        _CACHE[key] = _build_nc(reps, **kw)
    return _CACHE[key]


def _shard_inputs(x, Wq, bq, Wk, bk, Wv, bv):
    import ml_dtypes

    bf16 = ml_dtypes.bfloat16
    x2d = np.asarray(x, dtype=np.float32).reshape(T, HIDDEN)
    xTh = np.ascontiguousarray(x2d.T.astype(bf16))  # [HIDDEN, T]

    def wprep(W, sl):
        # [p, kt, c] with value W[sl][c, kt*128+p]
        ws = np.asarray(W, dtype=np.float32)[sl].astype(bf16)  # [128, 1024]
        return np.ascontiguousarray(ws.T.reshape(KT, P, P).transpose(1, 0, 2))

    in_maps = []
    for c in range(N_CORES):
        sl = slice(c * OPC, (c + 1) * OPC)
        in_maps.append({
            "xT": xTh,
            "wq": wprep(Wq, sl),
            "wk": wprep(Wk, sl),
            "wv": wprep(Wv, sl),
            "bq": np.ascontiguousarray(np.asarray(bq, dtype=np.float32)[sl].reshape(P, 1)),
            "bk": np.ascontiguousarray(np.asarray(bk, dtype=np.float32)[sl].reshape(P, 1)),
            "bv": np.ascontiguousarray(np.asarray(bv, dtype=np.float32)[sl].reshape(P, 1)),
        })
    return in_maps


def _gather(results):
    att = np.empty((B, NUM_HEADS, L, D_K), dtype=np.float32)
    for c in range(N_CORES):
        r = results[c]["out"]  # (HPC, D_K, T)
        for h in range(HPC):
            att[:, c * HPC + h] = r[h].T.reshape(B, L, D_K)
    return att


def run(x, Wq, bq, Wk, bk, Wv, bv, trace=False):
    from concourse.bass_utils import run_bass_kernel_spmd

    nc = get_nc()
    in_maps = _shard_inputs(x, Wq, bq, Wk, bk, Wv, bv)
    res = run_bass_kernel_spmd(
        nc, in_maps, core_ids=list(range(N_CORES)), trace=trace
    )
    return _gather(res.results), res


def kernel(x, Wq, bq, Wk, bk, Wv, bv):
    att, _ = run(x, Wq, bq, Wk, bk, Wv, bv, trace=False)
    return att
